# revision 16
# baseline (speedup 1.0000x reference)
"""DNN_WPE Trainium2 kernel: 8-core SPMD over frequency bins.

Per core: 132 (b,f) bins. PE builds per-bin covariances R (40x40 complex) and
cross-corr V, a batched bins-on-partitions Gauss-Jordan solves (R+eps)G=V on
the vector engines, PE computes the reverb tail, host reassembles.
"""
import sys, os
sys.path.insert(0, "/opt/trn_rl_repo")
import numpy as np
import ml_dtypes

import concourse.bass as bass
import concourse.bacc as bacc
import concourse.tile as tile
import concourse.mybir as mybir
from concourse.bass_utils import run_bass_kernel_spmd

F32 = mybir.dt.float32
BF16 = mybir.dt.bfloat16
ALU = mybir.AluOpType

B, T, C, F = 4, 512, 8, 257
TAPS, DELAY = 5, 3
EPS = 1e-6
WPE_EPS = 1e-10
NCORE = 8
FPAD = 264           # 8*33
FL = FPAD // NCORE   # 33 freqs per core
NB = B * FL          # 132 bins per core
KD = TAPS * C        # 40
NA = KD + C          # 48 augmented cols
TP = 520             # padded time for frame reads

_cache = {}


def ap(t_ap, offset_delta, free_dims):
    """Build an AP on the same tensor with explicit free dims (list of [step,count])."""
    a = t_ap[:, 0:1] if not isinstance(t_ap, bass.AP) else t_ap
    new = bass.AP(a.tensor, a.offset + offset_delta, [list(a.ap[0])] + [list(d) for d in free_dims])
    return new


def build_program():
    import os
    PH = os.environ.get("KERNEEL", os.environ.get("KERNEL_PHASES", "all"))
    nc = bacc.Bacc("TRN2", target_bir_lowering=False, debug=False)
    # inputs
    m_in = [nc.dram_tensor(f"m{c}", [128, NB * 96], BF16, kind="ExternalInput") for c in range(4)]
    ytre = nc.dram_tensor("ytre", [TP, NB * C], F32, kind="ExternalInput")
    ytim = nc.dram_tensor("ytim", [TP, NB * C], F32, kind="ExternalInput")
    phre = nc.dram_tensor("phre", [(NB // 2) * 128, T], F32, kind="ExternalInput")
    phim = nc.dram_tensor("phim", [(NB // 2) * 128, T], F32, kind="ExternalInput")
    # outputs
    tare = nc.dram_tensor("tare", [NB * C, T], F32, kind="ExternalOutput")
    taim = nc.dram_tensor("taim", [NB * C, T], F32, kind="ExternalOutput")
    pw = nc.dram_tensor("pw", [T, NB], F32, kind="ExternalOutput")

    with tile.TileContext(nc) as tc:
        import contextlib
        ctx = contextlib.ExitStack()
        with ctx:
            perm = ctx.enter_context(tc.tile_pool(name="perm", bufs=1))
            work = ctx.enter_context(tc.tile_pool(name="work", bufs=2))
            work1 = ctx.enter_context(tc.tile_pool(name="work1", bufs=1))
            psum = ctx.enter_context(tc.tile_pool(name="psum", bufs=1, space="PSUM"))
            drp = ctx.enter_context(tc.tile_pool(name="drp", bufs=2, space="DRAM"))
            
            # ---------- Phase 0: power + weights ----------
            wbf = [perm.tile([128, NB], BF16, tag=f"wbf{c}", name=f"wbf{c}") for c in range(4)]
            pwst = [perm.tile([128, NB], F32, tag=f"pwst{c}", name=f"pwst{c}") for c in range(4)]
            for half in range(2):          # 0: power out (rows 128c), 1: weights (rows 128c+7)
                for c in range(4):
                    yre_t = work1.tile([128, NB * C], F32, tag="p0re")
                    yim_t = work1.tile([128, NB * C], F32, tag="p0im")
                    r0 = 128 * c + 7 * half
                    nc.sync.dma_start(yre_t[:, :], ytre[r0:r0 + 128, :])
                    nc.sync.dma_start(yim_t[:, :], ytim[r0:r0 + 128, :])
                    sre = work1.tile([128, NB * C], F32, tag="p0sre")
                    sim = work1.tile([128, NB * C], F32, tag="p0sim")
                    nc.scalar.activation(sre[:, :], yre_t[:, :], mybir.ActivationFunctionType.Square)
                    nc.scalar.activation(sim[:, :], yim_t[:, :], mybir.ActivationFunctionType.Square)
                    ssum = work1.tile([128, NB * C], F32, tag="p0ss")
                    nc.vector.tensor_add(ssum[:, :], sre[:, :], sim[:, :])
                    red = work.tile([128, NB], F32, tag="p0red")
                    nc.vector.tensor_reduce(
                        red[:, :], ap(ssum, 0, [[C, NB], [1, C]]),
                        axis=mybir.AxisListType.X, op=ALU.add)
                    if half == 0:
                        nc.vector.tensor_scalar(pwst[c][:, :], red[:, :], 1.0 / C, EPS, ALU.mult, ALU.max)
                        nc.sync.dma_start(pw[128 * c:128 * (c + 1), :], pwst[c][:, :])
                    else:
                        pclamp = work.tile([128, NB], F32, tag="p0pc")
                        nc.vector.tensor_scalar(pclamp[:, :], red[:, :], 1.0 / C, EPS, ALU.mult, ALU.max)
                        winv = work.tile([128, NB], F32, tag="p0wi")
                        nc.vector.reciprocal(winv[:, :], pclamp[:, :])
                        nc.vector.tensor_copy(wbf[c][:, :], winv[:, :])

            if PH in ("p0",):
                return nc
            # ---------- Phase 1: R/P build on PE ----------
            ms1 = perm.tile([128, 2 * KD * NA], F32, tag="ms1")   # bins 0-119 on partitions
            ms2 = perm.tile([128, 2 * KD * NA], F32, tag="ms2")   # bins 120-131 on partitions 0-11
            rounds = [(0, 40), (40, 40), (80, 40), (120, 12)]
            for r0, nb in rounds:
                pt = [psum.tile([128, 512], F32, tag=f"ps{i}", name=f"ps{i}_{r0}") for i in range(8)]
                bases = [0, 0, 0, 0]
                msls, ats = [], []
                for c in range(4):
                    msl = work1.tile([128, 40 * 96], BF16, tag=f"msl{c}")
                    nc.sync.dma_start(msl[:, 0:nb * 96], m_in[c][:, r0 * 96:(r0 + nb) * 96])
                    at = work1.tile([128, 40 * 120], BF16, tag=f"at{c}")
                    wb = ap(wbf[c], r0, [[1, nb], [0, KD]])
                    nc.vector.tensor_mul(ap(at, 0, [[120, nb], [1, KD]]),
                                         ap(msl, 0, [[96, nb], [1, KD]]), wb)
                    nc.vector.tensor_mul(ap(at, KD, [[120, nb], [1, KD]]),
                                         ap(msl, NA, [[96, nb], [1, KD]]), wb)
                    nc.vector.tensor_scalar_mul(ap(at, 2 * KD, [[120, nb], [1, KD]]),
                                                ap(at, KD, [[120, nb], [1, KD]]), -1.0)
                    msls.append(msl); ats.append(at)
                for j in range(nb):
                    p = j // 10
                    s = j % 10
                    base = bases[p]
                    pre = pt[2 * p][base:base + KD, s * NA:(s + 1) * NA]
                    pim = pt[2 * p + 1][base:base + KD, s * NA:(s + 1) * NA]
                    for c in range(4):
                        msl, at = msls[c], ats[c]
                        a1 = at[:, j * 120:j * 120 + KD]
                        a2 = at[:, j * 120 + KD:j * 120 + 2 * KD]
                        a3 = at[:, j * 120 + 2 * KD:j * 120 + 3 * KD]
                        rre = msl[:, j * 96:j * 96 + NA]
                        rim = msl[:, j * 96 + NA:j * 96 + 2 * NA]
                        nc.tensor.matmul(pre, a1, rre, start=(c == 0), stop=False)
                        nc.tensor.matmul(pre, a2, rim, start=False, stop=(c == 3))
                        nc.tensor.matmul(pim, a1, rim, start=(c == 0), stop=False)
                        nc.tensor.matmul(pim, a3, rre, start=False, stop=(c == 3))
                # drain psum -> staging -> DMA-transpose into solve layout
                for p in range(4):
                    if p * 10 >= nb:
                        continue
                    nbp = min(10, nb - p * 10)
                    base = bases[p]
                    for plane in range(2):
                        stg = work.tile([128, 480], F32, tag="stg")
                        nc.scalar.copy(stg[base:base + KD, 0:nbp * NA], pt[2 * p + plane][base:base + KD, 0:nbp * NA])
                        gbin = r0 + p * 10
                        if gbin < 120:
                            dst, dbin = ms1, gbin
                        else:
                            dst, dbin = ms2, gbin - 120
                        dr = drp.tile([KD, 480], F32, tag="drb")
                        nc.sync.dma_start(dr[:, 0:nbp * NA], stg[base:base + KD, 0:nbp * NA])
                        dsrc = bass.AP(dr[0:1, 0:1].tensor, dr[0:1, 0:1].offset,
                                       [[NA, nbp], [480, KD], [1, NA]])
                        d = ap(dst[dbin:dbin + nbp, 0:1], plane * KD * NA, [[NA, KD], [1, NA]])
                        nc.sync.dma_start(d, dsrc)

            if PH == "p1":
                return nc
            # ---------- Phase 2: Gauss-Jordan on [R|V] ----------
            for ms, np_ in ((ms1, 120), (ms2, 12)):
                dg = ap(ms[0:np_, 0:1], 0, [[NA + 1, KD]])
                tr = work.tile([128, 1], F32, tag="gjtr")
                nc.vector.tensor_reduce(tr[0:np_, :], dg, axis=mybir.AxisListType.X, op=ALU.add)
                trs = work.tile([128, 1], F32, tag="gjtrs")
                nc.vector.tensor_scalar_mul(trs[0:np_, :], tr[0:np_, :], WPE_EPS / KD)
                nc.vector.tensor_scalar(ap(ms[0:np_, 0:1], 0, [[NA + 1, KD]]), dg,
                                        trs[0:np_, :], None, ALU.add)
                mure = work.tile([128, KD], F32, tag="gjmure")
                muim = work.tile([128, KD], F32, tag="gjmuim")
                scr = work.tile([128, 6 * NA], F32, tag="gjscr")
                upd = work.tile([128, 2 * KD * NA], F32, tag="gjupd")
                for k in range(KD):
                    ro = k * NA          # row offset in re plane
                    io = KD * NA + k * NA
                    W = NA - k - 1       # only columns right of pivot matter
                    co = k + 1
                    pr = ms[0:np_, ro + k:ro + k + 1]
                    pi = ms[0:np_, io + k:io + k + 1]
                    t1 = scr[0:np_, 0:1]; t2 = scr[0:np_, 1:2]
                    d = scr[0:np_, 2:3]; dn = scr[0:np_, 3:4]
                    cr = scr[0:np_, 4:5]; ci = scr[0:np_, 5:6]
                    nc.vector.tensor_mul(t1, pr, pr)
                    nc.vector.tensor_mul(t2, pi, pi)
                    nc.vector.tensor_add(d, t1, t2)
                    nc.vector.reciprocal(dn, d)
                    nc.vector.tensor_mul(cr, pr, dn)
                    nc.vector.tensor_scalar_mul(dn, dn, -1.0)
                    nc.vector.tensor_mul(ci, pi, dn)
                    # scale pivot row (cols right of pivot) in place
                    rre = ms[0:np_, ro + co:ro + NA]
                    rim = ms[0:np_, io + co:io + NA]
                    s1 = scr[0:np_, NA:NA + W]; s2 = scr[0:np_, 2 * NA:2 * NA + W]
                    s3 = scr[0:np_, 3 * NA:3 * NA + W]; s4 = scr[0:np_, 4 * NA:4 * NA + W]
                    nc.vector.tensor_scalar_mul(s1, rre, cr)
                    nc.vector.tensor_scalar_mul(s2, rim, ci)
                    nc.vector.tensor_scalar_mul(s3, rre, ci)
                    nc.vector.tensor_scalar_mul(s4, rim, cr)
                    nc.vector.tensor_sub(rre, s1, s2)
                    nc.vector.tensor_add(rim, s3, s4)
                    # multipliers = col k, masked at pivot
                    nc.vector.tensor_copy(mure[0:np_, :], ap(ms[0:np_, 0:1], k, [[NA, KD]]))
                    nc.vector.tensor_copy(muim[0:np_, :], ap(ms[0:np_, 0:1], KD * NA + k, [[NA, KD]]))
                    nc.vector.memset(mure[0:np_, k:k + 1], 0.0)
                    nc.vector.memset(muim[0:np_, k:k + 1], 0.0)
                    # rank-1 update on cols right of pivot: M -= mu (x) row
                    mre_b = ap(mure[0:np_, 0:1], 0, [[1, KD], [0, W]])
                    mim_b = ap(muim[0:np_, 0:1], 0, [[1, KD], [0, W]])
                    rre_b = ap(ms[0:np_, 0:1], ro + co, [[0, KD], [1, W]])
                    rim_b = ap(ms[0:np_, 0:1], io + co, [[0, KD], [1, W]])
                    mall_re = ap(ms[0:np_, 0:1], co, [[NA, KD], [1, W]])
                    mall_im = ap(ms[0:np_, 0:1], KD * NA + co, [[NA, KD], [1, W]])
                    u1 = ap(upd[0:np_, 0:1], 0, [[W, KD], [1, W]])
                    u2 = ap(upd[0:np_, 0:1], KD * NA, [[W, KD], [1, W]])
                    # re: M_re -= mu_re*row_re - mu_im*row_im
                    nc.vector.tensor_mul(u1, mre_b, rre_b)
                    nc.vector.tensor_sub(mall_re, mall_re, u1)
                    nc.gpsimd.tensor_mul(u2, mim_b, rim_b)
                    nc.vector.tensor_add(mall_re, mall_re, u2)
                    # im: M_im -= mu_re*row_im + mu_im*row_re
                    nc.gpsimd.tensor_mul(u1, mre_b, rim_b)
                    nc.vector.tensor_sub(mall_im, mall_im, u1)
                    nc.gpsimd.tensor_mul(u2, mim_b, rre_b)
                    nc.vector.tensor_sub(mall_im, mall_im, u2)

            if PH == "p2":
                return nc
            # ---------- Phase 3: gather filters G into PE layout ----------
            gt = perm.tile([128, NB * 24], F32, tag="gt")
            gneg1 = work.tile([128, KD * C], F32, tag="gneg1")
            gneg2 = work.tile([128, KD * C], F32, tag="gneg2")
            for ms, np_, b0 in ((ms1, 120, 0), (ms2, 12, 120)):
                gre = ap(ms[0:np_, 0:1], KD, [[NA, KD], [1, C]])
                gim = ap(ms[0:np_, 0:1], KD * NA + KD, [[NA, KD], [1, C]])
                nre = ap(gneg1[0:np_, 0:1], 0, [[C, KD], [1, C]])
                nim = ap(gneg2[0:np_, 0:1], 0, [[C, KD], [1, C]])
                nc.vector.tensor_scalar_mul(nre, gre, -1.0)
                nc.vector.tensor_scalar_mul(nim, gim, -1.0)
                for col, srcp in ((0, nre), (C, nim), (2 * C, gim)):
                    gd = drp.tile([128, KD * C], F32, tag="gdb")
                    nc.sync.dma_start(ap(gd[0:np_, 0:1], 0, [[C, KD], [1, C]]), srcp)
                    for rbase in (0, 64):
                        d = ap(gt[rbase:rbase + KD, 0:1], b0 * 24 + col, [[24, np_], [1, C]])
                        gsrc = bass.AP(gd[0:1, 0:1].tensor, gd[0:1, 0:1].offset,
                                       [[C, KD], [KD * C, np_], [1, C]])
                        nc.sync.dma_start(d, gsrc)

            if PH in ("p3",):
                return nc
            # ---------- Phase 4: tail matmuls ----------
            for g in range(NB // 4):
                pre = psum.tile([128, T], F32, tag=f"ps{2*(g%2)}", name=f"tre{g}")
                pim = psum.tile([128, T], F32, tag=f"ps{2*(g%2)+1}", name=f"tim{g}")
                scre = work.tile([128, T], F32, tag="tascre")
                scim = work.tile([128, T], F32, tag="tascim")
                for tloc in range(2):
                    tau = 2 * g + tloc
                    phr = work.tile([128, T], F32, tag="phr")
                    phi = work.tile([128, T], F32, tag="phi")
                    nc.sync.dma_start(phr[:, :], phre[tau * 128:(tau + 1) * 128, :])
                    nc.sync.dma_start(phi[:, :], phim[tau * 128:(tau + 1) * 128, :])
                    for loc in range(2):
                        bin_ = 4 * g + 2 * tloc + loc
                        rbase = 64 * loc
                        col = 32 * (2 * tloc + loc)
                        ggre = gt[rbase:rbase + KD, bin_ * 24:bin_ * 24 + C]
                        ggim = gt[rbase:rbase + KD, bin_ * 24 + C:bin_ * 24 + 2 * C]
                        ggpim = gt[rbase:rbase + KD, bin_ * 24 + 2 * C:bin_ * 24 + 3 * C]
                        rr = phr[rbase:rbase + KD, :]
                        ri = phi[rbase:rbase + KD, :]
                        nc.tensor.matmul(pre[col:col + C, :], ggre, rr, start=True, stop=False,
                                         tile_position=(rbase, col))
                        nc.tensor.matmul(pre[col:col + C, :], ggpim, ri, start=False, stop=True,
                                         tile_position=(rbase, col))
                        nc.tensor.matmul(pim[col:col + C, :], ggre, ri, start=True, stop=False,
                                         tile_position=(rbase, col))
                        nc.tensor.matmul(pim[col:col + C, :], ggim, rr, start=False, stop=True,
                                         tile_position=(rbase, col))
                nc.scalar.copy(scre[:, :], pre[:, :])
                nc.scalar.copy(scim[:, :], pim[:, :])
                for j in range(4):
                    row = (4 * g + j) * C
                    nc.sync.dma_start(tare[row:row + C, :], scre[32 * j:32 * j + C, :])
                    nc.sync.dma_start(taim[row:row + C, :], scim[32 * j:32 * j + C, :])
    nc.compile()
    return nc


def _prep_inputs(data_real, data_imag):
    """Host: pad F, transpose, build per-core input dicts."""
    dre = np.concatenate([data_real, data_real[..., :FPAD - F]], axis=-1)
    dim = np.concatenate([data_imag, data_imag[..., :FPAD - F]], axis=-1)
    # (B,T,C,Fp) -> (Fp,B,C,T)
    dre = np.ascontiguousarray(dre.transpose(3, 0, 2, 1)).astype(np.float32)
    dim = np.ascontiguousarray(dim.transpose(3, 0, 2, 1)).astype(np.float32)
    in_maps = []
    for i in range(NCORE):
        yre = dre[i * FL:(i + 1) * FL]          # (FL,B,C,T)
        yim = dim[i * FL:(i + 1) * FL]
        # bin = b*FL + fl
        yre = np.ascontiguousarray(yre.transpose(1, 0, 2, 3)).reshape(NB, C, T)
        yim = np.ascontiguousarray(yim.transpose(1, 0, 2, 3)).reshape(NB, C, T)
        pre = np.zeros((TP, NB, C), np.float32)
        pim = np.zeros((TP, NB, C), np.float32)
        pre[:T] = yre.transpose(2, 0, 1)
        pim[:T] = yim.transpose(2, 0, 1)
        mc = []
        for c in range(4):
            m = np.zeros((128, NB, 96), np.float32)
            for k in range(TAPS):
                m[:, :, k * C:(k + 1) * C] = pre[128 * c + 4 - k:128 * c + 4 - k + 128]
                m[:, :, NA + k * C:NA + (k + 1) * C] = pim[128 * c + 4 - k:128 * c + 4 - k + 128]
            m[:, :, KD:NA] = pre[128 * c + 7:128 * c + 7 + 128]
            m[:, :, NA + KD:2 * NA] = pim[128 * c + 7:128 * c + 7 + 128]
            if c == 3:
                m[121:] = 0.0          # frames t' >= 505 are invalid
            mc.append(m.reshape(128, NB * 96).astype(ml_dtypes.bfloat16))
        # phi tiles: 2 bins per 128 rows at bases 0,64; row (k,d) = Y[d, t-3-k]
        ph_re = np.zeros(((NB // 2) * 128, T), np.float32)
        ph_im = np.zeros(((NB // 2) * 128, T), np.float32)
        for k in range(TAPS):
            s = DELAY + k
            for loc in range(2):
                rows0 = loc * 64 + k * C
                ph_re.reshape(NB // 2, 128, T)[:, rows0:rows0 + C, s:] = yre[loc::2, :, :T - s]
                ph_im.reshape(NB // 2, 128, T)[:, rows0:rows0 + C, s:] = yim[loc::2, :, :T - s]
        in_maps.append({
            "m0": mc[0], "m1": mc[1], "m2": mc[2], "m3": mc[3],
            "ytre": pre.reshape(TP, NB * C), "ytim": pim.reshape(TP, NB * C),
            "phre": ph_re, "phim": ph_im,
        })
    return in_maps


def kernel(data_real, data_imag, ilens):
    if "nc" not in _cache:
        _cache["nc"] = build_program()
    nc = _cache["nc"]
    in_maps = _prep_inputs(np.asarray(data_real), np.asarray(data_imag))
    trace = bool(int(os.environ.get("KTRACE", "0")))
    res = run_bass_kernel_spmd(nc, in_maps, list(range(NCORE)), trace=trace)
    if res.exec_time_ns:
        _cache["exec_ns"] = res.exec_time_ns
    outs = res.results
    # reassemble
    enh_re = np.zeros((B, FPAD, C, T), np.float32)
    enh_im = np.zeros((B, FPAD, C, T), np.float32)
    power = np.zeros((B, FPAD, T), np.float32)
    for i in range(NCORE):
        ta_re = outs[i]["tare"].reshape(B, FL, C, T)
        ta_im = outs[i]["taim"].reshape(B, FL, C, T)
        pwc = outs[i]["pw"]  # (T, NB)
        enh_re[:, i * FL:(i + 1) * FL] = ta_re
        enh_im[:, i * FL:(i + 1) * FL] = ta_im
        power[:, i * FL:(i + 1) * FL] = pwc.T.reshape(B, FL, T)
    enh_re = enh_re[:, :F]
    enh_im = enh_im[:, :F]
    power = power[:, :F]
    # out = Y + (-tail); device returned -tail
    dre = np.asarray(data_real).transpose(0, 3, 2, 1)  # (B,F,C,T)
    dim = np.asarray(data_imag).transpose(0, 3, 2, 1)
    enh_re = dre + enh_re
    enh_im = dim + enh_im
    mask = (np.arange(T)[None, :] >= np.asarray(ilens)[:, None])[:, None, None, :]
    enh_re = np.where(mask, 0.0, enh_re).transpose(0, 3, 2, 1)
    enh_im = np.where(mask, 0.0, enh_im).transpose(0, 3, 2, 1)
    return enh_re.astype(np.float32), enh_im.astype(np.float32), power.astype(np.float32)
